# revision 14
# baseline (speedup 1.0000x reference)
"""Multi-head causal attention with RoPE on 8 Trainium2 NeuronCores.

Sharding: 8 cores = 2 (batch) x 4 (head groups of 4 heads).  Each core
computes its batch's attention for its 4 heads and the partial output
projection over those heads; the host sums the 4 partial outputs per batch.

Per-core kernel (all matmul operands bitcast to float32r: full PE rate at
~1.6e-4 relative error):
  - x^T resident in SBUF as 8 [128, 2048] tiles (m on partitions).
  - Q^T/K^T projection: lhsT = W^T chunks, rhs = x^T chunks -> [head_ch, pos].
  - RoPE: pair-swap permutation matmul on PE + 3 DVE elementwise ops.
  - V projection in natural [pos, head_ch] layout (lhsT = x^T chunks), stored
    with a fused ones column per head -> PV matmul also produces softmax
    denominators as row 0 of the PSUM accumulator.
  - Scores S^T[k, q] per (head-pair, q-chunk 512, k-tile 128): two heads run
    concurrently in the PE array (row groups 0-63 / 64-127), one ACT exp per
    block over both heads, causal diagonal handled by a triangular mask mul.
  - Normalization: reciprocal of denominators + selector-matrix matmul to
    broadcast them across partitions, one DVE mul.
  - Output projection: out^T[m, p] accumulated over the two 128-row
    head-pair chunks of O^T.
"""

import numpy as np
import sys

sys.path.insert(0, "/opt/trn_rl_repo")

import concourse.bass as bass
import concourse.tile as tile
from concourse import bacc, mybir
from concourse.bass_utils import run_bass_kernel_spmd

# Problem constants (hardcoded per contract).
B = 2
S = 2048
D_MODEL = 1024
N_HEADS = 16
D_HEAD = 64
HEADS_PER_CORE = 4
N_CORES = 8

F32 = mybir.dt.float32
F32R = mybir.dt.float32r
BF16 = mybir.dt.bfloat16

QC = 512          # q-chunk width
N_QC = S // QC    # 4
N_KT = S // 128   # 16 k-tiles
N_MC = D_MODEL // 128  # 8 m-chunks


def r(ap):
    """View an fp32 AP as float32r for full-rate PE matmuls."""
    return ap.bitcast(F32R)


def build_nc():
    nc = bacc.Bacc(None, target_bir_lowering=False)

    xT = nc.dram_tensor("xT", [D_MODEL, S], F32R, kind="ExternalInput")
    wqkT = nc.dram_tensor("wqkT", [D_MODEL, 512], F32R, kind="ExternalInput")
    wvT = nc.dram_tensor("wvT", [D_MODEL, 256], F32R, kind="ExternalInput")
    woT = nc.dram_tensor("woT", [256, D_MODEL], F32R, kind="ExternalInput")
    cosT = nc.dram_tensor("cosT", [128, S], F32, kind="ExternalInput")
    sinswapT = nc.dram_tensor("sinswapT", [128, S], F32, kind="ExternalInput")
    pswap = nc.dram_tensor("pswap", [128, 128], F32R, kind="ExternalInput")
    triA = nc.dram_tensor("triA", [128, 128], BF16, kind="ExternalInput")
    identB = nc.dram_tensor("identB", [128, 128], BF16, kind="ExternalInput")
    vinit = nc.dram_tensor("vinit", [128, 4], F32R, kind="ExternalInput")
    outT = nc.dram_tensor("outT", [D_MODEL, S], F32, kind="ExternalOutput")

    with tile.TileContext(nc) as tc:
        with (
            nc.allow_low_precision(reason="float32r matmul operands"),
            tc.tile_pool(name="consts", bufs=1) as consts,
            tc.tile_pool(name="persist", bufs=1) as persist,
        ):
            # ---- constant loads ----
            wqk_t = []
            wv_t = []
            for mc in range(N_MC):
                w1 = consts.tile([128, 512], F32R, tag=f"wqk{mc}")
                nc.sync.dma_start(w1[:], wqkT[mc * 128:(mc + 1) * 128, :])
                wqk_t.append(w1)
                w2 = consts.tile([128, 256], F32R, tag=f"wv{mc}")
                nc.sync.dma_start(w2[:], wvT[mc * 128:(mc + 1) * 128, :])
                wv_t.append(w2)
            wo_t = []
            for t in range(2):
                w3 = consts.tile([128, D_MODEL], F32R, tag=f"wo{t}")
                nc.sync.dma_start(w3[:], woT[t * 128:(t + 1) * 128, :])
                wo_t.append(w3)
            pswap_t = consts.tile([128, 128], F32R, tag="pswap")
            nc.sync.dma_start(pswap_t[:], pswap[:])
            triA_t = consts.tile([128, 128], BF16, tag="triA")
            nc.sync.dma_start(triA_t[:], triA[:])
            identB_t = consts.tile([128, 128], BF16, tag="identB")
            nc.sync.dma_start(identB_t[:], identB[:])

            # ---- persistent intermediates ----
            # QK tiles: index 0,1 = Q pair0/1; 2,3 = K pair0/1 (head pair t =
            # heads 2t, 2t+1; rows h*64+d in original channel order).
            qk_t = [persist.tile([128, S], F32R, tag=f"qk{i}", name=f"qk{i}") for i in range(4)]
            # V tiles, natural layout per k-tile: [128 pos, 4 heads, 64+1]
            # (last col of each head block is ones: the PV matmul emits O^T
            # in PSUM rows 0-63 and the softmax denominator in row 64).
            v_t = [persist.tile([128, 4, 65], F32R, tag=f"v{kt}", name=f"v{kt}") for kt in range(N_KT)]
            # O^T accumulators per head pair and softmax denominators.
            o_t = [persist.tile([128, S], F32R, tag=f"o{t}", name=f"o{t}") for t in range(2)]

            # ================= projection phase =================
            with (
                tc.tile_pool(name="xt", bufs=1) as xtp,
                tc.tile_pool(name="rope", bufs=2) as rope,
                tc.tile_pool(name="pps", bufs=2, space="PSUM") as pps,
                tc.tile_pool(name="rotps", bufs=2, space="PSUM") as rotps,
                tc.tile_pool(name="vps", bufs=2, space="PSUM") as vpsp,
            ):
                cos_t = xtp.tile([128, S], F32, tag="cos")
                nc.sync.dma_start(cos_t[:], cosT[:])
                sin_t = xtp.tile([128, S], F32, tag="sin")
                nc.sync.dma_start(sin_t[:], sinswapT[:])
                xt = []
                for mc in range(N_MC):
                    x1 = xtp.tile([128, S], F32R, tag=f"x{mc}")
                    nc.sync.dma_start(x1[:], xT[mc * 128:(mc + 1) * 128, :])
                    xt.append(x1)

                # Q^T / K^T with fused RoPE epilogue.
                for mt in range(4):
                    dest = qk_t[mt]
                    for pc in range(N_QC):
                        ps = pps.tile([128, QC], F32, tag="proj")
                        for mc in range(N_MC):
                            nc.tensor.matmul(
                                ps[:],
                                r(wqk_t[mc][:, mt * 128:(mt + 1) * 128]),
                                r(xt[mc][:, pc * QC:(pc + 1) * QC]),
                                start=(mc == 0),
                                stop=(mc == N_MC - 1),
                            )
                        csl = slice(pc * QC, (pc + 1) * QC)
                        # u = raw * sin_swapped; P @ u == rot(raw) * sin, so
                        # the pair-swap matmul absorbs the sin product and no
                        # raw PSUM->SBUF copy is needed.
                        u = rope.tile([128, QC], F32R, tag="u")
                        nc.vector.tensor_mul(u[:], ps[:], sin_t[:, csl])
                        rot = rotps.tile([128, QC], F32, tag="rot")
                        nc.tensor.matmul(rot[:], r(pswap_t[:]), r(u[:]),
                                         start=True, stop=True)
                        t1 = rope.tile([128, QC], F32, tag="t1")
                        nc.vector.tensor_mul(t1[:], ps[:], cos_t[:, csl])
                        nc.vector.tensor_add(dest[:, csl], t1[:], rot[:])

                # V in natural layout + ones columns.
                for kt in range(N_KT):
                    vp = vpsp.tile([128, 256], F32, tag="v")
                    for mc in range(N_MC):
                        nc.tensor.matmul(
                            vp[:],
                            r(xt[mc][:, kt * 128:(kt + 1) * 128]),
                            r(wv_t[mc][:]),
                            start=(mc == 0),
                            stop=(mc == N_MC - 1),
                        )
                    nc.sync.dma_start(
                        v_t[kt][:, :, 64:65],
                        vinit[:].rearrange("p (h o) -> p h o", o=1))
                    nc.vector.tensor_copy(
                        v_t[kt][:, :, 0:64],
                        vp.rearrange("p (h d) -> p h d", h=4),
                    )

            # ================= attention + output projection =================
            with (
                tc.tile_pool(name="pt", bufs=4) as ptp,
                tc.tile_pool(name="ostage", bufs=4) as ostage,
                tc.tile_pool(name="stg", bufs=4) as stgp,
                tc.tile_pool(name="bcp", bufs=2) as bcp,
                tc.tile_pool(name="sps", bufs=2, space="PSUM") as sps,
                tc.tile_pool(name="ops", bufs=1, space="PSUM") as opsp,
                tc.tile_pool(name="outps", bufs=2, space="PSUM") as outps,
            ):
                for qc in range(N_QC):
                    qsl = slice(qc * QC, (qc + 1) * QC)
                    for t in range(2):
                        q_tile = qk_t[t]
                        k_tile = qk_t[2 + t]
                        oacc = [opsp.tile([65, QC], F32, tag=f"oacc{_h}", name=f"oacc{_h}") for _h in range(2)]
                        nkt = (qc + 1) * 4
                        for kt in range(nkt):
                            j = kt - qc * 4
                            off = max(0, j) * 128
                            n = QC - off
                            sp = sps.tile([128, 2 * QC], F32, tag="scores")
                            diag = j >= 0
                            for h in range(2):
                                hsl = slice(h * 64, (h + 1) * 64)
                                nc.tensor.matmul(
                                    sp[:, h * QC + off: (h + 1) * QC],
                                    r(k_tile[hsl, kt * 128:(kt + 1) * 128]),
                                    r(q_tile[hsl, qc * QC + off:(qc + 1) * QC]),
                                    start=True, stop=not diag,
                                )
                                if diag:
                                    # add -1e9 above the causal diagonal:
                                    # triA.T @ I, accumulated into the psum
                                    nc.tensor.matmul(
                                        sp[:, h * QC + off: h * QC + off + 128],
                                        triA_t[:],
                                        identB_t[:],
                                        start=False, stop=True,
                                    )
                            pt = ptp.tile([128, 2 * QC], F32R, tag="pt")
                            sp2 = sp.rearrange("p (h q) -> p h q", h=2)
                            pt2 = pt.rearrange("p (h q) -> p h q", h=2)
                            nc.scalar.activation(
                                pt2[:, :, off:QC], sp2[:, :, off:QC],
                                mybir.ActivationFunctionType.Exp,
                            )
                            for h in range(2):
                                nc.tensor.matmul(
                                    oacc[h][:, off:QC],
                                    r(v_t[kt][:, 2 * t + h, :]),
                                    r(pt[:, h * QC + off:(h + 1) * QC]),
                                    start=(kt == 0),
                                    stop=(kt == nkt - 1),
                                )
                        for h in range(2):
                            rdr = stgp.tile([1, QC], F32, tag="rdr")
                            nc.scalar.copy(rdr[:], oacc[h][64:65, :])
                            rd = stgp.tile([1, QC], F32, tag="rd")
                            nc.vector.reciprocal_approx_fast(rd[:], rdr[:])
                            bc = bcp.tile([64, QC], F32, tag="bc")
                            nc.gpsimd.partition_broadcast(bc[:], rd[:])
                            nc.vector.tensor_mul(
                                o_t[t][h * 64:(h + 1) * 64, qsl],
                                oacc[h][0:64, :], bc[:])
                    # output projection for this q-chunk
                    for mt in range(N_MC):
                        op = outps.tile([128, QC], F32, tag="out")
                        for t in range(2):
                            nc.tensor.matmul(
                                op[:],
                                r(wo_t[t][:, mt * 128:(mt + 1) * 128]),
                                r(o_t[t][:, qsl]),
                                start=(t == 0), stop=(t == 1),
                            )
                        st = ostage.tile([128, QC], F32, tag="st")
                        if mt % 2 == 0:
                            nc.vector.tensor_copy(st[:], op[:])
                        else:
                            nc.scalar.copy(st[:], op[:])
                        nc.sync.dma_start(
                            outT[mt * 128:(mt + 1) * 128, qsl], st[:])

    nc.compile()
    return nc


def make_in_maps(x, key_weight, query_weight, value_weight, output_weight,
                 sines, cosines):
    """Host-side sharding + layout prep. Returns list of 8 per-core dicts."""
    f32 = np.float32

    # RoPE factor tiles [128, S]: row r (within a 64-channel head block)
    # carries cos/sin of pair index (r % 64) // 2; sin rows get sign -1 on
    # even rows (out_even = e*c - o*s) and +1 on odd rows.
    idx = np.tile(np.repeat(np.arange(D_HEAD // 2), 2), 2)  # [128]
    sign = np.tile(np.array([-1.0, 1.0], dtype=f32), 64)
    cosT = np.ascontiguousarray(cosines.T[idx, :]).astype(f32)          # [128, S]
    sinT = sines.T[idx, :] * sign[:, None]
    # rows pre-permuted by the pair swap so that P @ (x * sinswapT) equals
    # rot(x) * sinT
    rr128 = np.arange(128) ^ 1
    sinswapT = np.ascontiguousarray(sinT[rr128, :]).astype(f32)

    psw = np.zeros((128, 128), dtype=f32)
    rr = np.arange(128)
    psw[rr, rr ^ 1] = 1.0

    import ml_dtypes
    bf16 = ml_dtypes.bfloat16
    # mask matmul: (triA.T @ I)[k, q] = triA[q, k] = -1e9 where k > q
    triA = np.where(np.arange(128)[None, :] > np.arange(128)[:, None],
                    np.float32(-1e9), np.float32(0.0)).astype(bf16)
    identB = np.eye(128, dtype=np.float32).astype(bf16)
    vinit_np = np.ones((128, 4), dtype=f32)

    in_maps = []
    for c in range(N_CORES):
        b, g = divmod(c, 4)
        hs = slice(g * HEADS_PER_CORE, (g + 1) * HEADS_PER_CORE)
        xTb = np.ascontiguousarray(x[b].T).astype(f32)
        wqT = np.ascontiguousarray(
            query_weight[hs].transpose(2, 0, 1).reshape(D_MODEL, 256)).astype(f32)
        wkT = np.ascontiguousarray(
            key_weight[hs].transpose(2, 0, 1).reshape(D_MODEL, 256)).astype(f32)
        wvT = np.ascontiguousarray(
            value_weight[hs].transpose(2, 0, 1).reshape(D_MODEL, 256)).astype(f32)
        woT = np.ascontiguousarray(
            output_weight[:, hs, :].transpose(1, 2, 0).reshape(256, D_MODEL)
        ).astype(f32)
        in_maps.append({
            "xT": xTb,
            "wqkT": np.concatenate([wqT, wkT], axis=1),
            "wvT": wvT,
            "woT": woT,
            "cosT": cosT,
            "sinswapT": sinswapT,
            "pswap": psw,
            "triA": triA,
            "identB": identB,
            "vinit": vinit_np,
        })
    return in_maps


_NC_CACHE = None


def get_nc():
    global _NC_CACHE
    if _NC_CACHE is None:
        _NC_CACHE = build_nc()
    return _NC_CACHE


def kernel(x, key_weight, query_weight, value_weight, output_weight,
           sines, cosines, _trace=False, _trace_kwargs=None):
    in_maps = make_in_maps(x, key_weight, query_weight, value_weight,
                           output_weight, sines, cosines)
    nc = get_nc()
    kw = {}
    if _trace:
        kw = dict(trace=True, **(_trace_kwargs or {}))
    res = run_bass_kernel_spmd(nc, in_maps, core_ids=list(range(N_CORES)), **kw)
    out = np.zeros((B, S, D_MODEL), dtype=np.float32)
    for c in range(N_CORES):
        b = c // 4
        out[b] += res.results[c]["outT"].T
    kernel.last_result = res
    return out


# revision 15
# speedup vs baseline: 1.1200x; 1.1200x over previous
"""Multi-head causal attention with RoPE on 8 Trainium2 NeuronCores.

Sharding: 8 cores = 2 (batch) x 4 (head groups of 4 heads).  Each core
computes its batch's attention for its 4 heads and the partial output
projection over those heads; the host sums the 4 partial outputs per batch.

Per-core kernel (all matmul operands bitcast to float32r: full PE rate at
~1.6e-4 relative error):
  - x^T resident in SBUF as 8 [128, 2048] tiles (m on partitions).
  - Q^T/K^T projection: lhsT = W^T chunks, rhs = x^T chunks -> [head_ch, pos].
  - RoPE: pair-swap permutation matmul on PE + 3 DVE elementwise ops.
  - V projection in natural [pos, head_ch] layout (lhsT = x^T chunks), stored
    with a fused ones column per head -> PV matmul also produces softmax
    denominators as row 0 of the PSUM accumulator.
  - Scores S^T[k, q] per (head-pair, q-chunk 512, k-tile 128): two heads run
    concurrently in the PE array (row groups 0-63 / 64-127), one ACT exp per
    block over both heads, causal diagonal handled by a triangular mask mul.
  - Normalization: reciprocal of denominators + selector-matrix matmul to
    broadcast them across partitions, one DVE mul.
  - Output projection: out^T[m, p] accumulated over the two 128-row
    head-pair chunks of O^T.
"""

import numpy as np
import sys

sys.path.insert(0, "/opt/trn_rl_repo")

import concourse.bass as bass
import concourse.tile as tile
from concourse import bacc, mybir
from concourse.bass_utils import run_bass_kernel_spmd

# Problem constants (hardcoded per contract).
B = 2
S = 2048
D_MODEL = 1024
N_HEADS = 16
D_HEAD = 64
HEADS_PER_CORE = 4
N_CORES = 8

F32 = mybir.dt.float32
F32R = mybir.dt.float32r
BF16 = mybir.dt.bfloat16

QC = 512          # q-chunk width
N_QC = S // QC    # 4
N_KT = S // 128   # 16 k-tiles
N_MC = D_MODEL // 128  # 8 m-chunks


def r(ap):
    """View an fp32 AP as float32r for full-rate PE matmuls."""
    return ap.bitcast(F32R)


def build_nc():
    nc = bacc.Bacc(None, target_bir_lowering=False)

    xT = nc.dram_tensor("xT", [D_MODEL, S], F32R, kind="ExternalInput")
    wqkT = nc.dram_tensor("wqkT", [D_MODEL, 512], F32R, kind="ExternalInput")
    wvT = nc.dram_tensor("wvT", [D_MODEL, 256], F32R, kind="ExternalInput")
    woT = nc.dram_tensor("woT", [256, D_MODEL], F32R, kind="ExternalInput")
    cosT = nc.dram_tensor("cosT", [128, S], F32, kind="ExternalInput")
    sinswapT = nc.dram_tensor("sinswapT", [128, S], F32, kind="ExternalInput")
    pswap = nc.dram_tensor("pswap", [128, 128], F32R, kind="ExternalInput")
    triA = nc.dram_tensor("triA", [128, 128], BF16, kind="ExternalInput")
    identB = nc.dram_tensor("identB", [128, 128], BF16, kind="ExternalInput")
    vinit = nc.dram_tensor("vinit", [128, 4], F32R, kind="ExternalInput")
    outT = nc.dram_tensor("outT", [D_MODEL, S], F32, kind="ExternalOutput")

    with tile.TileContext(nc) as tc:
        with (
            nc.allow_low_precision(reason="float32r matmul operands"),
            tc.tile_pool(name="consts", bufs=1) as consts,
            tc.tile_pool(name="persist", bufs=1) as persist,
        ):
            # ---- constant loads ----
            wqk_t = []
            wv_t = []
            for mc in range(N_MC):
                w1 = consts.tile([128, 512], F32R, tag=f"wqk{mc}")
                nc.sync.dma_start(w1[:], wqkT[mc * 128:(mc + 1) * 128, :])
                wqk_t.append(w1)
                w2 = consts.tile([128, 256], F32R, tag=f"wv{mc}")
                nc.sync.dma_start(w2[:], wvT[mc * 128:(mc + 1) * 128, :])
                wv_t.append(w2)
            wo_t = []
            for t in range(2):
                w3 = consts.tile([128, D_MODEL], F32R, tag=f"wo{t}")
                nc.sync.dma_start(w3[:], woT[t * 128:(t + 1) * 128, :])
                wo_t.append(w3)
            pswap_t = consts.tile([128, 128], F32R, tag="pswap")
            nc.sync.dma_start(pswap_t[:], pswap[:])
            triA_t = consts.tile([128, 128], BF16, tag="triA")
            nc.sync.dma_start(triA_t[:], triA[:])
            identB_t = consts.tile([128, 128], BF16, tag="identB")
            nc.sync.dma_start(identB_t[:], identB[:])

            # ---- persistent intermediates ----
            # QK tiles: index 0,1 = Q pair0/1; 2,3 = K pair0/1 (head pair t =
            # heads 2t, 2t+1; rows h*64+d in original channel order).
            qk_t = [persist.tile([128, S], F32R, tag=f"qk{i}", name=f"qk{i}") for i in range(4)]
            # V tiles, natural layout per k-tile: [128 pos, 4 heads, 64+1]
            # (last col of each head block is ones: the PV matmul emits O^T
            # in PSUM rows 0-63 and the softmax denominator in row 64).
            v_t = [persist.tile([128, 4, 65], F32R, tag=f"v{kt}", name=f"v{kt}") for kt in range(N_KT)]
            # O^T accumulators per head pair and softmax denominators.
            o_t = [persist.tile([128, S], F32R, tag=f"o{t}", name=f"o{t}") for t in range(2)]

            # ================= projection phase =================
            with (
                tc.tile_pool(name="xt", bufs=1) as xtp,
                tc.tile_pool(name="rope", bufs=2) as rope,
                tc.tile_pool(name="pps", bufs=2, space="PSUM") as pps,
                tc.tile_pool(name="rotps", bufs=2, space="PSUM") as rotps,
                tc.tile_pool(name="vps", bufs=2, space="PSUM") as vpsp,
            ):
                cos_t = xtp.tile([128, S], F32, tag="cos")
                sin_t = xtp.tile([128, S], F32, tag="sin")
                xt = [xtp.tile([128, S], F32R, tag=f"x{mc}", name=f"xt{mc}")
                      for mc in range(N_MC)]
                # pc-major DMA order so the first projection chains (which
                # accumulate over all m-chunks of one 512-column slice) can
                # start long before the full 8 MB of x^T has landed.
                for pc in range(N_QC):
                    csl = slice(pc * QC, (pc + 1) * QC)
                    for mc in range(N_MC):
                        nc.sync.dma_start(xt[mc][:, csl], xT[mc * 128:(mc + 1) * 128, csl])
                    nc.sync.dma_start(cos_t[:, csl], cosT[:, csl])
                    nc.sync.dma_start(sin_t[:, csl], sinswapT[:, csl])

                # Q^T / K^T with fused RoPE epilogue.
                for pc in range(N_QC):
                    for mt in range(4):
                        dest = qk_t[mt]
                        ps = pps.tile([128, QC], F32, tag="proj")
                        for mc in range(N_MC):
                            nc.tensor.matmul(
                                ps[:],
                                r(wqk_t[mc][:, mt * 128:(mt + 1) * 128]),
                                r(xt[mc][:, pc * QC:(pc + 1) * QC]),
                                start=(mc == 0),
                                stop=(mc == N_MC - 1),
                            )
                        csl = slice(pc * QC, (pc + 1) * QC)
                        # u = raw * sin_swapped; P @ u == rot(raw) * sin, so
                        # the pair-swap matmul absorbs the sin product and no
                        # raw PSUM->SBUF copy is needed.
                        u = rope.tile([128, QC], F32R, tag="u")
                        nc.vector.tensor_mul(u[:], ps[:], sin_t[:, csl])
                        rot = rotps.tile([128, QC], F32, tag="rot")
                        nc.tensor.matmul(rot[:], r(pswap_t[:]), r(u[:]),
                                         start=True, stop=True)
                        t1 = rope.tile([128, QC], F32, tag="t1")
                        nc.vector.tensor_mul(t1[:], ps[:], cos_t[:, csl])
                        nc.vector.tensor_add(dest[:, csl], t1[:], rot[:])

                # V in natural layout + ones columns.
                for kt in range(N_KT):
                    vp = vpsp.tile([128, 256], F32, tag="v")
                    for mc in range(N_MC):
                        nc.tensor.matmul(
                            vp[:],
                            r(xt[mc][:, kt * 128:(kt + 1) * 128]),
                            r(wv_t[mc][:]),
                            start=(mc == 0),
                            stop=(mc == N_MC - 1),
                        )
                    nc.sync.dma_start(
                        v_t[kt][:, :, 64:65],
                        vinit[:].rearrange("p (h o) -> p h o", o=1))
                    nc.vector.tensor_copy(
                        v_t[kt][:, :, 0:64],
                        vp.rearrange("p (h d) -> p h d", h=4),
                    )

            # ================= attention + output projection =================
            with (
                tc.tile_pool(name="pt", bufs=4) as ptp,
                tc.tile_pool(name="ostage", bufs=4) as ostage,
                tc.tile_pool(name="stg", bufs=4) as stgp,
                tc.tile_pool(name="bcp", bufs=2) as bcp,
                tc.tile_pool(name="sps", bufs=2, space="PSUM") as sps,
                tc.tile_pool(name="ops", bufs=1, space="PSUM") as opsp,
                tc.tile_pool(name="outps", bufs=2, space="PSUM") as outps,
            ):
                for qc in range(N_QC):
                    qsl = slice(qc * QC, (qc + 1) * QC)
                    for t in range(2):
                        q_tile = qk_t[t]
                        k_tile = qk_t[2 + t]
                        oacc = [opsp.tile([65, QC], F32, tag=f"oacc{_h}", name=f"oacc{_h}") for _h in range(2)]
                        nkt = (qc + 1) * 4
                        for kt in range(nkt):
                            j = kt - qc * 4
                            off = max(0, j) * 128
                            n = QC - off
                            sp = sps.tile([128, 2 * QC], F32, tag="scores")
                            diag = j >= 0
                            for h in range(2):
                                hsl = slice(h * 64, (h + 1) * 64)
                                nc.tensor.matmul(
                                    sp[:, h * QC + off: (h + 1) * QC],
                                    r(k_tile[hsl, kt * 128:(kt + 1) * 128]),
                                    r(q_tile[hsl, qc * QC + off:(qc + 1) * QC]),
                                    start=True, stop=not diag,
                                )
                            if diag:
                                # add -1e9 above the causal diagonal:
                                # triA.T @ I, accumulated into the psum
                                for h in range(2):
                                    nc.tensor.matmul(
                                        sp[:, h * QC + off: h * QC + off + 128],
                                        triA_t[:],
                                        identB_t[:],
                                        start=False, stop=True,
                                    )
                            pt = ptp.tile([128, 2 * QC], F32R, tag="pt")
                            sp2 = sp.rearrange("p (h q) -> p h q", h=2)
                            pt2 = pt.rearrange("p (h q) -> p h q", h=2)
                            nc.scalar.activation(
                                pt2[:, :, off:QC], sp2[:, :, off:QC],
                                mybir.ActivationFunctionType.Exp,
                            )
                            for h in range(2):
                                nc.tensor.matmul(
                                    oacc[h][:, off:QC],
                                    r(v_t[kt][:, 2 * t + h, :]),
                                    r(pt[:, h * QC + off:(h + 1) * QC]),
                                    start=(kt == 0),
                                    stop=(kt == nkt - 1),
                                )
                        for h in range(2):
                            rdr = stgp.tile([1, QC], F32, tag="rdr")
                            nc.scalar.copy(rdr[:], oacc[h][64:65, :])
                            rd = stgp.tile([1, QC], F32, tag="rd")
                            nc.vector.reciprocal_approx_fast(rd[:], rdr[:])
                            bc = bcp.tile([64, QC], F32, tag="bc")
                            nc.gpsimd.partition_broadcast(bc[:], rd[:])
                            nc.vector.tensor_mul(
                                o_t[t][h * 64:(h + 1) * 64, qsl],
                                oacc[h][0:64, :], bc[:])
                    # output projection for this q-chunk
                    for mt in range(N_MC):
                        op = outps.tile([128, QC], F32, tag="out")
                        for t in range(2):
                            nc.tensor.matmul(
                                op[:],
                                r(wo_t[t][:, mt * 128:(mt + 1) * 128]),
                                r(o_t[t][:, qsl]),
                                start=(t == 0), stop=(t == 1),
                            )
                        st = ostage.tile([128, QC], F32, tag="st")
                        if mt % 2 == 0:
                            nc.vector.tensor_copy(st[:], op[:])
                        else:
                            nc.scalar.copy(st[:], op[:])
                        nc.sync.dma_start(
                            outT[mt * 128:(mt + 1) * 128, qsl], st[:])

    nc.compile()
    return nc


def make_in_maps(x, key_weight, query_weight, value_weight, output_weight,
                 sines, cosines):
    """Host-side sharding + layout prep. Returns list of 8 per-core dicts."""
    f32 = np.float32

    # RoPE factor tiles [128, S]: row r (within a 64-channel head block)
    # carries cos/sin of pair index (r % 64) // 2; sin rows get sign -1 on
    # even rows (out_even = e*c - o*s) and +1 on odd rows.
    idx = np.tile(np.repeat(np.arange(D_HEAD // 2), 2), 2)  # [128]
    sign = np.tile(np.array([-1.0, 1.0], dtype=f32), 64)
    cosT = np.ascontiguousarray(cosines.T[idx, :]).astype(f32)          # [128, S]
    sinT = sines.T[idx, :] * sign[:, None]
    # rows pre-permuted by the pair swap so that P @ (x * sinswapT) equals
    # rot(x) * sinT
    rr128 = np.arange(128) ^ 1
    sinswapT = np.ascontiguousarray(sinT[rr128, :]).astype(f32)

    psw = np.zeros((128, 128), dtype=f32)
    rr = np.arange(128)
    psw[rr, rr ^ 1] = 1.0

    import ml_dtypes
    bf16 = ml_dtypes.bfloat16
    # mask matmul: (triA.T @ I)[k, q] = triA[q, k] = -1e9 where k > q
    triA = np.where(np.arange(128)[None, :] > np.arange(128)[:, None],
                    np.float32(-1e9), np.float32(0.0)).astype(bf16)
    identB = np.eye(128, dtype=np.float32).astype(bf16)
    vinit_np = np.ones((128, 4), dtype=f32)

    in_maps = []
    for c in range(N_CORES):
        b, g = divmod(c, 4)
        hs = slice(g * HEADS_PER_CORE, (g + 1) * HEADS_PER_CORE)
        xTb = np.ascontiguousarray(x[b].T).astype(f32)
        wqT = np.ascontiguousarray(
            query_weight[hs].transpose(2, 0, 1).reshape(D_MODEL, 256)).astype(f32)
        wkT = np.ascontiguousarray(
            key_weight[hs].transpose(2, 0, 1).reshape(D_MODEL, 256)).astype(f32)
        wvT = np.ascontiguousarray(
            value_weight[hs].transpose(2, 0, 1).reshape(D_MODEL, 256)).astype(f32)
        woT = np.ascontiguousarray(
            output_weight[:, hs, :].transpose(1, 2, 0).reshape(256, D_MODEL)
        ).astype(f32)
        in_maps.append({
            "xT": xTb,
            "wqkT": np.concatenate([wqT, wkT], axis=1),
            "wvT": wvT,
            "woT": woT,
            "cosT": cosT,
            "sinswapT": sinswapT,
            "pswap": psw,
            "triA": triA,
            "identB": identB,
            "vinit": vinit_np,
        })
    return in_maps


_NC_CACHE = None


def get_nc():
    global _NC_CACHE
    if _NC_CACHE is None:
        _NC_CACHE = build_nc()
    return _NC_CACHE


def kernel(x, key_weight, query_weight, value_weight, output_weight,
           sines, cosines, _trace=False, _trace_kwargs=None):
    in_maps = make_in_maps(x, key_weight, query_weight, value_weight,
                           output_weight, sines, cosines)
    nc = get_nc()
    kw = {}
    if _trace:
        kw = dict(trace=True, **(_trace_kwargs or {}))
    res = run_bass_kernel_spmd(nc, in_maps, core_ids=list(range(N_CORES)), **kw)
    out = np.zeros((B, S, D_MODEL), dtype=np.float32)
    for c in range(N_CORES):
        b = c // 4
        out[b] += res.results[c]["outT"].T
    kernel.last_result = res
    return out
